# revision 14
# baseline (speedup 1.0000x reference)
"""Trainium2 Bass kernel for nn_ContextualLSTM (B=32, S=1024, E=512, H=256, V=50000).

Strategy (8 NeuronCores, SPMD):
  - Data-parallel over batch: core c owns samples 4c..4c+3 and runs BOTH LSTM
    directions (two independent recurrence chains pipeline on each core).
  - Host does only integer index preprocessing (token reversal, compaction
    ranks), weight packing (transposed fp16 layouts) and the final concat.
  - Device phases per core:
      P1: indirect-DMA embedding gather (fp16 rows) -> PE transpose ->
          input-projection matmuls (W_ih stationary fp16) -> xg preactivations
          stored fp16 in SBUF (gate-unit-major, per 128-step block).
      P2: 1024-step recurrence. Per step/dir: xg injected into PSUM via an
          identity matmul (start=True), 16 W_hh-stationary fp16 matmuls
          accumulate h-feedback; sigmoid/tanh on ScalarE read PSUM directly;
          cell/hidden updates on VectorE. Hidden state kept fp16 for the
          matmul feed, fp32 for the cell state and outputs.
      P3: incremental PE transpose of the hidden history + masked-compaction
          scatter (indirect DMA) straight into the pre-zeroed outputs.
"""
import os
from contextlib import ExitStack

import numpy as np

import concourse.bass as bass
import concourse.tile as tile
from concourse import mybir
from concourse.bass_utils import run_bass_kernel_spmd
from concourse.masks import make_identity

V, E, H = 50000, 512, 256
B_FULL, S_FULL = 32, 1024
NB = 4                 # samples per core
NCORES = 8
F16 = mybir.dt.float16
F32 = mybir.dt.float32
I32 = mybir.dt.int32
AF = mybir.ActivationFunctionType

# gate-chunk permutation: pytorch rows [i(0:256) f(256:512) g(512:768) o(768:1024)]
# device mslot order [i0 i1 f0 f1 o0 o1 g0 g1]
P_ROWS = np.concatenate([
    np.arange(0, 256), np.arange(256, 512),
    np.arange(768, 1024), np.arange(512, 768),
])


# ---------------------------------------------------------------- host prep --

def _pack_hh(W, dt):
    lhsT = W[P_ROWS].T.astype(dt)                       # (256, 1024)
    cols = [lhsT[k * 128:(k + 1) * 128, m * 128:(m + 1) * 128]
            for m in range(8) for k in range(2)]
    return np.ascontiguousarray(np.concatenate(cols, axis=1))   # (128, 2048)


def _pack_ih(W, dt):
    lhsT = W[P_ROWS].T.astype(dt)                       # (512, 1024)
    cols = [lhsT[ke * 128:(ke + 1) * 128, m * 128:(m + 1) * 128]
            for m in range(8) for ke in range(4)]
    return np.ascontiguousarray(np.concatenate(cols, axis=1))   # (128, 4096)


def _pack_bias(bih, bhh):
    b = (np.asarray(bih, np.float32) + np.asarray(bhh, np.float32))[P_ROWS]
    return np.ascontiguousarray(b.reshape(8, 128).T)    # (128, 8)


def host_prep_core(c, inputs, seq_length, fmask, w, S):
    """in_map dict for core c (weights packed fp16, indices int32)."""
    bs = slice(NB * c, NB * c + NB)
    inp = np.asarray(inputs)[bs].astype(np.int64)
    L = np.asarray(seq_length)[bs].astype(np.int64)
    fm = np.asarray(fmask)[bs].astype(bool)
    nblk = S // 128

    tf = inp.T.reshape(-1).astype(np.int32)             # (S*NB,), j = t*NB+b
    t_idx = np.arange(S)[None, :]
    rev_t = np.where(t_idx < L[:, None], L[:, None] - 1 - t_idx, 0)
    rev_inp = np.where(t_idx < L[:, None],
                       np.take_along_axis(inp, rev_t, axis=1), 0)
    tb = rev_inp.T.reshape(-1).astype(np.int32)
    ntile = S * NB // 128
    toki_f = np.ascontiguousarray(tf.reshape(ntile, 128).T)     # (128, ntile)
    toki_b = np.ascontiguousarray(tb.reshape(ntile, 128).T)

    counts = fm.sum(axis=1)
    rank = np.cumsum(fm, axis=1) - 1
    DUMP = NB * S
    fdest = np.where(fm, np.arange(NB)[:, None] * S + rank, DUMP).astype(np.int32)
    bdest = np.where(fm, np.arange(NB)[:, None] * S + (counts[:, None] - 1 - rank),
                     DUMP).astype(np.int32)

    def pack_idx(dest):
        out = np.zeros((128, NB * nblk), np.int32)
        for b in range(NB):
            for wi in range(nblk):
                out[:, b * nblk + wi] = dest[b, wi * 128:(wi + 1) * 128]
        return out

    return dict(
        toki_f=toki_f, toki_b=toki_b,
        whh_f=_pack_hh(np.asarray(w["f_Whh"]), np.float16),
        whh_b=_pack_hh(np.asarray(w["b_Whh"]), np.float16),
        wih_f=_pack_ih(np.asarray(w["f_Wih"]), np.float16),
        wih_b=_pack_ih(np.asarray(w["b_Wih"]), np.float16),
        bias_f=_pack_bias(w["f_bih"], w["f_bhh"]),
        bias_b=_pack_bias(w["b_bih"], w["b_bhh"]),
        idxf=pack_idx(fdest), idxb=pack_idx(bdest),
    )


# ------------------------------------------------------------- bass program --

def build_bass(S=S_FULL):
    nblk = S // 128
    nc = bass.Bass()

    emb16 = nc.declare_dram_parameter("emb16", [V, E], F16, isOutput=False)
    prm = {}
    for d in ("f", "b"):
        prm[f"toki_{d}"] = nc.declare_dram_parameter(
            f"toki_{d}", [128, S * NB // 128], I32, isOutput=False)
        prm[f"whh_{d}"] = nc.declare_dram_parameter(
            f"whh_{d}", [128, 2048], F16, isOutput=False)
        prm[f"wih_{d}"] = nc.declare_dram_parameter(
            f"wih_{d}", [128, 4096], F16, isOutput=False)
        prm[f"bias_{d}"] = nc.declare_dram_parameter(
            f"bias_{d}", [128, 8], F32, isOutput=False)
    idxf_d = nc.declare_dram_parameter("idxf", [128, NB * nblk], I32, isOutput=False)
    idxb_d = nc.declare_dram_parameter("idxb", [128, NB * nblk], I32, isOutput=False)
    outf = nc.declare_dram_parameter("outf", [NB * S + 1, 256], F32, isOutput=True)
    outb = nc.declare_dram_parameter("outb", [NB * S + 1, 256], F32, isOutput=True)
    outx = {"f": outf, "b": outb}

    with tile.TileContext(nc) as tc, ExitStack() as ctx:
        const = ctx.enter_context(tc.tile_pool(name="const", bufs=1))
        xg_pool = ctx.enter_context(tc.tile_pool(name="xg", bufs=1))
        gath = ctx.enter_context(tc.tile_pool(name="gath", bufs=4))
        xtp = ctx.enter_context(tc.tile_pool(name="xtp", bufs=2))
        psx = ctx.enter_context(tc.tile_pool(name="psx", bufs=2, space="PSUM"))
        psp = ctx.enter_context(tc.tile_pool(name="psp", bufs=2, space="PSUM"))
        psg = ctx.enter_context(tc.tile_pool(name="psg", bufs=3, space="PSUM"))
        pst = ctx.enter_context(tc.tile_pool(name="pst", bufs=1, space="PSUM"))
        ew = ctx.enter_context(tc.tile_pool(name="ew", bufs=3))
        stg = ctx.enter_context(tc.tile_pool(name="stg", bufs=2))
        rowp = ctx.enter_context(tc.tile_pool(name="rowp", bufs=3))

        # ---- constants / weights to SBUF
        id16 = const.tile([128, 128], F16, tag="id16", name="id16")
        make_identity(nc, id16[:])
        id32 = const.tile([128, 128], F32, tag="id32", name="id32")
        make_identity(nc, id32[:])

        sb = {}
        for d in ("f", "b"):
            for nm, shp, dt in (("whh", [128, 2048], F16),
                                ("wih", [128, 4096], F16),
                                ("bias", [128, 8], F32),
                                ("toki", [128, S * NB // 128], I32)):
                t = const.tile(shp, dt, tag=f"{nm}_{d}", name=f"{nm}_{d}")
                nc.sync.dma_start(out=t[:], in_=prm[f"{nm}_{d}"][:])
                sb[f"{nm}_{d}"] = t
        idx_sb = {}
        for d, src in (("f", idxf_d), ("b", idxb_d)):
            t = const.tile([128, NB * nblk], I32, tag=f"idx_{d}", name=f"idx_{d}")
            nc.sync.dma_start(out=t[:], in_=src[:])
            idx_sb[d] = t

        # ---- zero-fill outputs (rows j >= counts must read as zeros)
        ztile = const.tile([128, 512], F32, tag="ztile", name="ztile")
        nc.vector.memset(ztile[:], 0.0)
        nz = NB * S // 128
        for od in (outf, outb):
            oview = od[0:NB * S, :].rearrange("(a p) n -> p a n", p=128)
            for z0 in range(0, nz, 2):
                zn = min(2, nz - z0)
                nc.sync.dma_start(
                    out=oview[:, z0:z0 + zn, :],
                    in_=ztile[:].rearrange("p (a n) -> p a n", n=256)[:, :zn, :])

        # ---- persistent state
        h16 = {d: const.tile([128, 8], F16, tag=f"h16_{d}", name=f"h16_{d}") for d in "fb"}
        cst = {d: const.tile([128, 8], F32, tag=f"c_{d}", name=f"c_{d}") for d in "fb"}
        for d in "fb":
            nc.vector.memset(h16[d][:], 0.0)
            nc.vector.memset(cst[d][:], 0.0)

        xg_tiles = {d: [xg_pool.tile([128, 128 * 32], F16, tag=f"xg_{d}_{i}", name=f"xg_{d}_{i}")
                        for i in range(nblk)] for d in "fb"}

        # ---------------------------------------------------------- phase 1 --
        def phase1_block(d, blk):
            xT = [xtp.tile([128, 512], F16, tag=f"xT{ke}", name=f"xT{ke}") for ke in range(4)]
            for g in range(4):
                xrow = gath.tile([128, E], F16, tag="xrow", name="xrow")
                nc.gpsimd.indirect_dma_start(
                    out=xrow[:], out_offset=None,
                    in_=emb16[:],
                    in_offset=bass.IndirectOffsetOnAxis(
                        ap=sb[f"toki_{d}"][:, blk * 4 + g: blk * 4 + g + 1],
                        axis=0),
                )
                for ke in range(4):
                    pxt = psx.tile([128, 128], F16, tag="pxt", name="pxt")
                    nc.tensor.transpose(
                        pxt[:], xrow[:, ke * 128:(ke + 1) * 128], id16[:])
                    if ke % 2 == 0:
                        nc.scalar.copy(xT[ke][:, g * 128:(g + 1) * 128], pxt[:])
                    else:
                        nc.vector.tensor_copy(xT[ke][:, g * 128:(g + 1) * 128], pxt[:])
            xgt = xg_tiles[d][blk]
            for ms in range(8):
                pp = psp.tile([128, 512], F32, tag="pp", name="pp")
                for ke in range(4):
                    nc.tensor.matmul(
                        pp[:],
                        lhsT=sb[f"wih_{d}"][:, (ms * 4 + ke) * 128:
                                            (ms * 4 + ke + 1) * 128],
                        rhs=xT[ke][:],
                        start=(ke == 0), stop=(ke == 3))
                out_ap = xgt[:].rearrange("p (t x) -> p t x", x=32)[:, :, ms * 4:(ms + 1) * 4]
                in_ap = pp[:].rearrange("p (t b) -> p t b", b=4)
                nc.scalar.activation(out_ap, in_ap, AF.Identity,
                                     bias=sb[f"bias_{d}"][:, ms:ms + 1])

        # ---------------------------------------------------------- phase 2 --
        stage_cur = {}

        def step(d, t):
            xgt = xg_tiles[d][t // 128]
            xg_slice = xgt[:, (t % 128) * 32:(t % 128) * 32 + 32]
            pg = psg.tile([128, 32], F32, tag="pg", name=f"pg_{d}")
            nc.tensor.matmul(pg[:], lhsT=id16[:], rhs=xg_slice,
                             start=True, stop=False)
            for ms in range(8):
                for k in range(2):
                    nc.tensor.matmul(
                        pg[:, ms * 4:(ms + 1) * 4],
                        lhsT=sb[f"whh_{d}"][:, (ms * 2 + k) * 128:
                                            (ms * 2 + k + 1) * 128],
                        rhs=h16[d][:, k * 4:(k + 1) * 4],
                        start=False, stop=(ms == 7 and k == 1))
            sig = ew.tile([128, 24], F32, tag=f"sig_{d}", name=f"sig_{d}")
            tg = ew.tile([128, 8], F32, tag=f"tg_{d}", name=f"tg_{d}")
            nc.scalar.activation(sig[:], pg[:, 0:24], AF.Sigmoid)
            nc.scalar.activation(tg[:], pg[:, 24:32], AF.Tanh)
            t1 = ew.tile([128, 8], F32, tag=f"t1_{d}", name=f"t1_{d}")
            t2 = ew.tile([128, 8], F32, tag=f"t2_{d}", name=f"t2_{d}")
            nc.vector.tensor_mul(t1[:], sig[:, 8:16], cst[d][:])
            nc.vector.tensor_mul(t2[:], sig[:, 0:8], tg[:])
            nc.vector.tensor_add(cst[d][:], t1[:], t2[:])
            tcv = ew.tile([128, 8], F32, tag=f"tc_{d}", name=f"tc_{d}")
            nc.scalar.activation(tcv[:], cst[d][:], AF.Tanh)
            nc.vector.tensor_mul(h16[d][:], sig[:, 16:24], tcv[:])
            nc.vector.tensor_mul(
                stage_cur[d][:, (t % 128) * 8:(t % 128) * 8 + 8],
                sig[:, 16:24], tcv[:])

        # ---------------------------------------------------------- phase 3 --
        def phase3_window(d, w):
            st = stage_cur[d]
            for b in range(NB):
                row = rowp.tile([128, 256], F32, tag="row", name="row")
                for kc in range(2):
                    pt = pst.tile([128, 128], F32, tag="pt", name="pt")
                    in_ap = st[:].rearrange("p (t x) -> p x t", x=8)[:, kc * 4 + b, :]
                    nc.tensor.transpose(pt[:], in_ap, id32[:])
                    if kc == 0:
                        nc.scalar.copy(row[:, kc * 128:(kc + 1) * 128], pt[:])
                    else:
                        nc.vector.tensor_copy(row[:, kc * 128:(kc + 1) * 128], pt[:])
                nc.gpsimd.indirect_dma_start(
                    out=outx[d][:], out_offset=bass.IndirectOffsetOnAxis(
                        ap=idx_sb[d][:, b * nblk + w: b * nblk + w + 1], axis=0),
                    in_=row[:], in_offset=None)

        # ---- emit program (interleaved so recurrence chases projection)
        emitted = 0
        for d in "fb":
            phase1_block(d, 0)
        if nblk > 1:
            for d in "fb":
                phase1_block(d, 1)
        for w in range(nblk):
            for d in "fb":
                stage_cur[d] = stg.tile([128, 1024], F32, tag=f"stage_{d}", name=f"stage_{d}")
            for tl in range(128):
                t = w * 128 + tl
                step("f", t)
                step("b", t)
            for d in "fb":
                phase3_window(d, w)
            if w + 2 < nblk:
                for d in "fb":
                    phase1_block(d, w + 2)

    return nc


# -------------------------------------------------------------- entry point --

def kernel(inputs, seq_length, fmask, bmask, out_seq_length,
           emb, f_Wih, f_Whh, f_bih, f_bhh, b_Wih, b_Whh, b_bih, b_bhh):
    S = int(np.asarray(inputs).shape[1])
    w = dict(f_Wih=f_Wih, f_Whh=f_Whh, f_bih=f_bih, f_bhh=f_bhh,
             b_Wih=b_Wih, b_Whh=b_Whh, b_bih=b_bih, b_bhh=b_bhh)
    emb16 = np.ascontiguousarray(np.asarray(emb).astype(np.float16))

    nc = build_bass(S)
    in_maps = []
    for c in range(NCORES):
        m = host_prep_core(c, inputs, seq_length, fmask, w, S)
        m["emb16"] = emb16
        in_maps.append(m)

    res = run_bass_kernel_spmd(nc, in_maps, list(range(NCORES)))
    out = np.zeros((B_FULL, S, 2 * H), np.float32)
    for c in range(NCORES):
        r = res.results[c]
        out[NB * c:NB * c + NB] = np.concatenate(
            [r["outf"][:NB * S].reshape(NB, S, 256),
             r["outb"][:NB * S].reshape(NB, S, 256)], axis=2)
    return out[:, :int(out_seq_length)]
